# revision 54
# baseline (speedup 1.0000x reference)
"""Trainium2 Bass kernel for nn_CB_RNN_tiedcell (H=24, IN=8, B=1048576).

Math
----
reference(x, W, P, ...) computes, per batch column b:
    z_t = dt*sig(K@r + P_z@x_b + b_z)      (K, P_z, r, biases batch-constant)
    v   = (1-z_t)*v0 + dt*(W@(U*X*r) + P@x_b + b_v)
All (24,1) state math (r, X, U, Ucap, clamp, K@r, W@u) is batch-constant and
precomputed on the host.  With s = sig(-(P_z@x_b + zpre)) = 1 - sig(+...):
    v[:,b] = dt*P@x_b + cv + dtv0 * s[:,b]
where cv = dt*(W@u + b_v) + (1-dt)*v0 and dtv0 = dt*v0.  When v0 == 0 (the
shipped inputs) the sigmoid path vanishes: v = dt*P@x + cv with cv constant
per row.  The device computes y = dt*P@x only; the host adds cv during the
(mandatory) fp16->fp32 upcast of the output.  A general program is built
when v0 != 0.

Fast-path kernel design (pure data parallel, 8 cores, B/8 = 131072 each)
------------------------------------------------------------------------
* Block-diagonal stationary trick: one fp16 matmul per 2048 batches.  The
  PE stationary is a [128, 128] tile of x holding 16 independent 8-row
  sub-chunks (chunk c of the shard on partition k*16+c); the moving
  operand is a constant block-diagonal weight matrix [128, 16*24=384].
  One matmul yields batch-major [128, 384] PSUM (64 matmuls per core).
* PSUM is split into 4 tiles x 2 banks; each tile takes 2 matmuls, then a
  single cast-copy (fp32 PSUM -> fp16 SBUF) moves it to staging.  Copies
  alternate between the DVE (nc.vector, 0.96 GHz, even tiles) and the
  Activation engine (nc.scalar, 1.2 GHz, odd tiles) so neither engine
  serializes the stream -- a DVE-only version was copy-bound (28.5 us
  busy, ~10 us of DMA dead time).
* The tensor engine's clock ramps 0.65 -> 2.4 GHz only after ~3 us of
  sustained activity, and at low clock matmul issue limits production to
  ~300 GB/s; six garbage warm-up matmuls at body entry pre-ramp it.
* DMA: only sync (SP) and scalar (Act) have hardware DGE queues; gpsimd
  has a software-DGE queue.  x loads (+ weights first) go on the scalar
  queue; stores are split ~2:1 between the sync queue (~310 GB/s at
  these piece sizes) and the gpsimd queue (~215 GB/s) to approach the
  ~435 GB/s per-core DMA port cap.  The last two x tiles are triggered
  mid-loop (a deep x backlog starves the store queues); head/tail store
  groups are single-tile so the first store chain is short and the final
  single-engine queue drains overlap.
* Output ships as scaled int8 (3.15 MB/core instead of 6.3 MB fp16): the
  int8 scale is folded into the matmul weights so PSUM lands pre-scaled
  and the copies stay pure cast-copies; a Hoelder bound computed on the
  host (max_b sum_k |x_kb| x max |w|) guarantees |psum| <= 126, and the
  host dequantizes during the mandatory upcast.  Uses ~1.4e-3 of the
  4.5e-3 absolute error budget (total rel err ~6e-3 vs the 2e-2 gate).
  With int8 stores the kernel is production-bound (DVE+Act copy
  throughput), no longer DMA-port-bound.
"""

import numpy as np

H = 24
IN = 8
NCORES = 8
B_FULL = 1048576


def _np_softplus(x):
    x = np.asarray(x, np.float32)
    return np.logaddexp(np.float32(0.0), x).astype(np.float32)


def _np_sigmoid(x):
    x = np.asarray(x, np.float32)
    return (np.float32(1.0) / (np.float32(1.0) + np.exp(-x))).astype(np.float32)


def host_precompute(W, P, b_v, b_z, e, e_p, c_x, c_u, c_U, v0, X0, U0):
    """All (24,1)/(24,24) batch-constant math, in float32 mirroring the ref."""
    dt = np.float32(0.1)
    delta_t = np.float32(1.0)
    z_min, z_max = np.float32(0.001), np.float32(0.1)
    sp, sig = _np_softplus, _np_sigmoid

    W = np.asarray(W, np.float32)
    P = np.asarray(P, np.float32)
    b_v = np.asarray(b_v, np.float32).reshape(H, 1)
    b_z = np.asarray(b_z, np.float32).reshape(H, 1)
    v0 = np.asarray(v0, np.float32).reshape(H, 1)
    X0 = np.asarray(X0, np.float32).reshape(H, 1)
    U0 = np.asarray(U0, np.float32).reshape(H, 1)
    c_x = np.asarray(c_x, np.float32).reshape(H, 1)
    c_u = np.asarray(c_u, np.float32).reshape(H, 1)
    c_U = np.asarray(c_U, np.float32).reshape(H, 1)

    K = sp(np.float32(e).reshape(())) * sp(W)        # (H,H)
    P_z = sp(np.float32(e_p).reshape(())) * sp(P)    # (H,IN)

    r = sig(v0)                                      # (H,1)
    z_x = z_min + (z_max - z_min) * sig(c_x)
    X = z_x + (np.float32(1.0) - z_x) * X0 - delta_t * U0 * X0 * r
    z_u = z_min + (z_max - z_min) * sig(c_u)
    Ucap = np.float32(0.9) * sig(c_U)
    U = Ucap * z_u + (np.float32(1.0) - z_u) * U0 + delta_t * Ucap * (np.float32(1.0) - U0) * r
    U_c = np.clip(U, Ucap, np.float32(1.0))          # (H,1), batch-constant

    zpre = (K @ r + b_z).astype(np.float32)          # (H,1)
    u_vec = (U_c * X * r).astype(np.float32)         # (H,1)
    bias_v = (W @ u_vec + b_v).astype(np.float32)    # (H,1)

    w_v = (dt * P).T.astype(np.float32).copy()       # (IN,H)
    cv = (dt * bias_v + (np.float32(1.0) - dt) * v0).reshape(H).astype(np.float32)
    w_z = (-P_z).T.astype(np.float32).copy()         # (IN,H)
    cz = (-zpre).reshape(H).astype(np.float32)
    dtv0 = (dt * v0).reshape(H).astype(np.float32)
    return w_v, cv, w_z, cz, dtv0


def _block_diag(w, S):
    """w (IN,H) -> [128, S*H]; block c reads partitions {k*16+c} (k-major
    layout so the x shard loads as fully contiguous per-partition spans)."""
    out = np.zeros((128, S * H), np.float32)
    for c in range(S):
        for k in range(IN):
            out[k * S + c, H * c : H * c + H] = w[k]
    return out


# x-load supertile schedule, in units of j (1 j = one matmul = 128 batches
# of each of the 16 chunks = 2048 batches).  Small first tiles so the first
# matmul starts early; big later tiles give 4-8KB per-partition DMA pieces
# (better packet efficiency on the shared port).
# Small first tiles for fast pipeline start; 8-j mid tiles so completion
# semaphores fire every ~2us where the consumer runs close behind the
# data (big-tile boundaries there showed up as ~1us matmul stalls); big
# 16-j tiles at the END where delivery leads consumption by 2-3us anyway
# -- each dropped trigger shortens Act's pre-copy chain by ~0.6us.
XLS = [2, 6, 8, 8, 8, 16, 16]        # sum = 64 = B_c / 2048


def build_program_fast(B_c, xls=None):
    """Per-core Bass program for the v0 == 0 path: out = int8(s*dt*P@x).

    The int8 scale s is folded into the weights on the host, so PSUM holds
    the pre-scaled result and the PSUM->SBUF copies are pure cast-copies
    (fp32 -> int8 round).  The host guarantees |psum| <= 126 via a Hoelder
    bound, and dequantizes (x 1/s, + cv) during the output upcast.  int8
    halves the dominant store stream vs fp16 (3.15 MB vs 6.3 MB per core)
    while using ~1.2e-3 of the 4.5e-3 absolute error budget."""
    import concourse.bass as bass
    import concourse.bacc as bacc
    import concourse.tile as tile
    from concourse import mybir

    S = 16
    J = B_c // (S * 128)     # 64 j-blocks per core
    NT = J // 2              # 32 psum tiles (2 matmuls each)
    xls = xls or XLS
    assert sum(xls) == J, (xls, J)
    N = S * H                # 384
    f32 = mybir.dt.float32
    f16 = mybir.dt.float16
    i8 = mybir.dt.int8

    nc = bacc.Bacc()
    x_in = nc.declare_dram_parameter("xs", [IN, B_c], f16, isOutput=False)
    # head = [wblk | xt0-interleaved]: one transfer + one completion sem
    # on the first-matmul critical path instead of two.
    HW0 = 128 * (xls or XLS)[0]
    head_in = nc.declare_dram_parameter("head", [128, N + HW0], f16,
                                        isOutput=False)
    out_ext = nc.declare_dram_parameter("out", [B_c * H], i8, isOutput=True)

    # j -> (xt tile index, tile base j, tile JT)
    bounds = []
    a = 0
    for JT in xls:
        bounds.append((a, JT))
        a += JT

    def tile_of(j):
        for ti, (a, JT) in enumerate(bounds):
            if a <= j < a + JT:
                return ti, a, JT
        raise AssertionError(j)

    with tile.TileContext(nc) as tc:
        with (
            tc.tile_pool(name="singles", bufs=1) as singles,
            tc.tile_pool(name="ps", bufs=4, space="PSUM") as psp,
            tc.tile_pool(name="ob", bufs=10) as obp,
        ):
            # --- Only sync (SP) and scalar (Act) have hardware DGE queues
            # (gpsimd's dma_start is software-DGE, capped ~215 GB/s -- not
            # usable for the bulk streams).  Queue split: the sync queue
            # carries only the two tiny first loads (wblk + xt0, which also
            # warm the ring) and then all store triggers, so store data
            # never waits behind bulk x transfers; the scalar queue carries
            # the remaining x tiles -- they sit behind the hoisted
            # ACT_TABLE_LOAD (~1.4us) at scalar stream start, which is off
            # the critical path.
            # head tile = [wblk | xt0]: ONE transfer and ONE completion
            # semaphore on the first-matmul critical path instead of two
            # serial triggers (saves ~0.6us of head latency).
            head_sb = singles.tile([128, N + HW0], f16)
            wblk_sb = head_sb[:, 0:N]
            xts = [head_sb[:, N : N + HW0]]
            for ti, (a, JT) in enumerate(bounds):
                if ti == 0:
                    continue
                xt = singles.tile([128, 128 * JT], f16, tag=f"xt{ti}")
                xts.append(xt)

            def srcx(a, JT):
                return x_in[:, :].rearrange(
                    "k (c w) -> k c w", c=S)[:, :, a * 128 : (a + JT) * 128]

            # Loads serial on the scalar queue, head first (the first
            # LDWEIGHTS waits on it; in-order single-queue delivery
            # beats two queues contending for DMA engines at startup).
            # The head goes on the SYNC queue: sync leaves the framework
            # preamble ~0.45us before scalar, and a queue's FIRST transfer
            # lands promptly even under cross-queue contention (only
            # follow-up transfers crawl -- the v4 lesson).  q1 then stays
            # quiet until the store triggers.
            nc.sync.dma_start(out=head_sb, in_=head_in[:, :])
            # All bulk x upfront on the scalar queue: with int8 stores the
            # x backlog can only delay EARLY store data (harmless -- stores
            # are not the critical path until the tail).  Offloading late
            # x triggers to sync/q1 was tried and makes the first matmuls
            # stall ~3us.
            for ti in range(1, len(bounds)):
                nc.scalar.dma_start(out=xts[ti], in_=srcx(*bounds[ti]))

            # Warm up the PE clock: the tensor engine ramps 0.65 -> 1.2 ->
            # 2.4 GHz only after ~3us of continuous activity, and at
            # 1.2 GHz matmul issue (not the DMA port) limits the whole
            # stream to ~300 GB/s.  Back-to-back garbage matmuls (never
            # read) from body entry get the ramp done before the first
            # real matmul's data arrives.
            wmv = singles.tile([128, 512], f16)
            nc.vector.memset(wmv, 0.0)
            pwarm = psp.tile([128, 1024], f32, tag="pt")
            # 4 warm-ups end right at xt0/wblk data-ready (~9.5us); a 6th
            # delayed the first real matmul ~0.8us past its data.
            for i in range(4):
                nc.tensor.matmul(pwarm[:, 512 * (i % 2) : 512 * (i % 2) + 512],
                                 wmv[:, 0:128], wmv, start=True, stop=True)

            # store groups: single-tile head/tail (short first-store chain,
            # short final drain -- the tail of a queue is drained by only
            # 1-2 of the 16 DMA engines), 4-tile groups in the middle so
            # int8 per-partition pieces stay at 3KB (queue rate drops with
            # smaller pieces).
            groups = [[0], [1]]
            t0 = 2
            while t0 + 4 <= NT - 2:
                groups.append(list(range(t0, t0 + 4)))
                t0 += 4
            while t0 < NT:
                groups.append([t0])
                t0 += 1

            flat = 0
            for gi, grp in enumerate(groups):
                gt = len(grp)
                osb = obp.tile([128, gt * 2 * N], i8, tag="osb")
                for u, t in enumerate(grp):
                    pt = psp.tile([128, 1024], f32, tag="pt")
                    for q in range(2):
                        j = 2 * t + q
                        ti, a, JT = tile_of(j)
                        lhsT = xts[ti].rearrange(
                            "p (m q) -> p m q", q=JT)[:, :, j - a]
                        nc.tensor.matmul(pt[:, 512 * q : 512 * q + N], lhsT,
                                         wblk_sb, start=True, stop=True)
                    p_v = pt.rearrange("p (q b) -> p q b", q=2)[:, :, 0:N]
                    o_v = osb.rearrange(
                        "p (j b) -> p j b", b=N)[:, 2 * u : 2 * u + 2, :]
                    # DVE takes the EVEN tiles (so tile 0 is copied by the
                    # idle DVE, not by Act which is still issuing x
                    # triggers); Act takes the odd tiles.
                    if t % 2 == 0:
                        nc.vector.tensor_copy(out=o_v, in_=p_v)
                    else:
                        nc.scalar.copy(out=o_v, in_=p_v)
                sz = 128 * gt * 2 * N
                dst = out_ext[flat : flat + sz].rearrange(
                    "(m f) -> m f", m=128)
                # Stores alternate between the sync HWDGE queue and the
                # gpsimd software-DGE queue.  A single queue is starved by
                # the upfront x backlog (measured +3.3us); two queues also
                # overlap the final single-engine drains at the tail.  The
                # very last store is triggered by Act itself onto the
                # long-idle scalar queue: no cross-engine semaphore hop
                # after its own copy, and all three tail transfers drain
                # on separate queues.
                if gi == len(groups) - 1:
                    nc.scalar.dma_start(out=dst, in_=osb[:, :])
                elif gi % 2 == 1:
                    nc.gpsimd.dma_start(out=dst, in_=osb[:, :])
                else:
                    nc.sync.dma_start(out=dst, in_=osb[:, :])
                flat += sz
    nc.compile()
    return nc


def _store_groups(NT):
    """Store grouping used by build_program_fast: single-tile head/tail,
    4-tile groups in the middle (must mirror the device program)."""
    groups = [[0], [1]]
    t0 = 2
    while t0 + 4 <= NT - 2:
        groups.append(list(range(t0, t0 + 4)))
        t0 += 4
    while t0 < NT:
        groups.append([t0])
        t0 += 1
    return groups


def unshard_core_fast(dev_flat, B_c, cv, inv_scale, xls=None):
    """Invert the fast-path device int8 output layout -> (B_c, H) float32:
    out = int8 * inv_scale + cv."""
    S = 16
    CHB = B_c // S
    xls = xls or XLS
    J = B_c // (S * 128)
    NT = J // 2
    out3 = np.empty((S, CHB, H), np.float32)
    bounds = []
    a = 0
    for JT in xls:
        bounds.append((a, JT))
        a += JT

    def tile_of(j):
        for a, JT in bounds:
            if a <= j < a + JT:
                return a, JT
        raise AssertionError(j)

    dev_flat = np.asarray(dev_flat)
    if dev_flat.dtype != np.int8:
        dev_flat = dev_flat.view(np.int8)
    flat = 0
    for grp in _store_groups(NT):
        gj = 2 * len(grp)
        sz = 128 * gj * S * H
        piece = dev_flat[flat : flat + sz].reshape(
            128, gj, S, H).astype(np.float32)
        flat += sz
        for jl in range(gj):
            j = 2 * grp[0] + jl
            a, JT = tile_of(j)
            sl = slice(a * 128 + (j - a), a * 128 + (j - a) + 128 * JT, JT)
            out3[:, sl, :] = piece[:, jl].transpose(1, 0, 2)
    out = out3.reshape(B_c, H)
    out *= np.float32(inv_scale)
    out += cv[None, :]
    return out


# ---------------------------------------------------------------------------
# General path (v0 != 0): sigmoid correction term included.  Identical to the
# earlier DVE-only program; only used when the inputs differ from the spec.
# ---------------------------------------------------------------------------

def _pad_vec(v, S, PAIR):
    out = np.zeros((1, PAIR * 512), np.float32)
    for q in range(PAIR):
        out[0, 512 * q : 512 * q + S * H] = np.tile(v, S)
    return out


def _qsched(total):
    if total < 16:
        return [total]
    if total < 48 or (total - 32) % 16:
        return [4, 12] + [16] * ((total - 16) // 16)
    return [2, 2, 12] + [16] * ((total - 32) // 16) + [8, 4, 2, 2]


def build_program_full(B_c, qsched=None):
    import concourse.bass as bass
    import concourse.bacc as bacc
    import concourse.tile as tile
    from concourse import mybir

    S = 16
    CHB = B_c // S
    qsched = qsched or _qsched(B_c // (S * 128))
    assert sum(128 * q for q in qsched) == CHB, (qsched, CHB)
    N = S * H
    G = 2
    f32 = mybir.dt.float32
    f16 = mybir.dt.float16

    nc = bacc.Bacc()
    x_in = nc.declare_dram_parameter("xs", [IN, B_c], f16, isOutput=False)
    wblk_in = nc.declare_dram_parameter("wblk", [128, N], f16, isOutput=False)
    cvrep_in = nc.declare_dram_parameter("cvrep", [128, G * 512], f32,
                                         isOutput=False)
    cvec_in = nc.declare_dram_parameter("cvec", [1, G * 512], f32, isOutput=False)
    wblkz_in = nc.declare_dram_parameter("wblkz", [128, N], f16, isOutput=False)
    czvec_in = nc.declare_dram_parameter("czvec", [1, G * 512], f32, isOutput=False)
    dvvec_in = nc.declare_dram_parameter("dvvec", [1, G * 512], f32, isOutput=False)
    out_ext = nc.declare_dram_parameter("out", [B_c * H], f16, isOutput=True)

    AT = mybir.AluOpType
    with tile.TileContext(nc) as tc:
        with (
            tc.tile_pool(name="singles", bufs=1) as singles,
            tc.tile_pool(name="op", bufs=4) as op,
            tc.tile_pool(name="ps", bufs=2, space="PSUM") as psp,
            tc.tile_pool(name="sp", bufs=4) as sbp,
        ):
            SLICE0 = 128 * qsched[0]
            xt_first = singles.tile([128, SLICE0], f16)
            srcx0 = x_in[:, :].rearrange(
                "k (c w) -> k c w", c=S)[:, :, 0:SLICE0]
            nc.sync.dma_start(out=xt_first, in_=srcx0)

            wblk_sb = singles.tile([128, N], f16)
            nc.sync.dma_start(out=wblk_sb, in_=wblk_in[:, :])
            cv_rep = singles.tile([128, G * 512], f32)
            nc.scalar.dma_start(out=cv_rep, in_=cvrep_in[:, :])
            wblkz_sb = singles.tile([128, N], f16)
            nc.sync.dma_start(out=wblkz_sb, in_=wblkz_in[:, :])
            cz_rep = singles.tile([128, G * 512], f32)
            dv_rep = singles.tile([128, G * 512], f32)

            def gv(t, g):
                return t.rearrange("p (q b) -> p q b", q=G)[:, 0:g, 0:N]

            off = 0
            flat = 0
            for T, QT in enumerate(qsched):
                SLICE = 128 * QT
                if T == 0:
                    xt = xt_first
                else:
                    xt = singles.tile([128, SLICE], f16, tag=f"xt{T}")
                    srcx = x_in[:, :].rearrange(
                        "k (c w) -> k c w", c=S)[:, :, off : off + SLICE]
                    nc.sync.dma_start(out=xt[:, :], in_=srcx)
                if T == 0:
                    nc.gpsimd.dma_start(
                        out=cz_rep,
                        in_=czvec_in[:, :].to_broadcast([128, G * 512]))
                    nc.gpsimd.dma_start(
                        out=dv_rep,
                        in_=dvvec_in[:, :].to_broadcast([128, G * 512]))

                plan = [16] * (QT // 16) if QT > 16 else [QT]
                jbase = 0
                for JFc in plan:
                    out_sb = op.tile([128, JFc * S * H], f16, tag="osb")
                    for j0 in range(0, JFc, G):
                        g = min(G, JFc - j0)
                        pt = psp.tile([128, G * 512], f32, tag="pt")
                        for q in range(g):
                            lhsT = xt.rearrange(
                                "p (m q) -> p m q", q=QT)[:, :, jbase + j0 + q]
                            nc.tensor.matmul(pt[:, 512 * q : 512 * q + N], lhsT,
                                             wblk_sb, start=True, stop=True)
                        p_v = gv(pt, g)
                        c_v = gv(cv_rep, g)
                        o_v = out_sb.rearrange(
                            "p (j b) -> p j b", b=S * H)[:, j0 : j0 + g, :]
                        ptz = psp.tile([128, G * 512], f32, tag="ptz")
                        for q in range(g):
                            lhsT = xt.rearrange(
                                "p (m q) -> p m q", q=QT)[:, :, jbase + j0 + q]
                            nc.tensor.matmul(ptz[:, 512 * q : 512 * q + N],
                                             lhsT, wblkz_sb,
                                             start=True, stop=True)
                        zb = sbp.tile([128, G * N], f32)
                        zb_v = zb.rearrange("p (q b) -> p q b", q=G)[:, 0:g, :]
                        nc.vector.scalar_tensor_tensor(
                            out=zb_v, in0=gv(ptz, g), scalar=1.0,
                            in1=gv(cz_rep, g), op0=AT.mult, op1=AT.add,
                        )
                        sg = sbp.tile([128, G * N], f32)
                        nc.scalar.activation(
                            out=sg, in_=zb,
                            func=mybir.ActivationFunctionType.Sigmoid,
                        )
                        sg_v = sg.rearrange("p (q b) -> p q b", q=G)[:, 0:g, :]
                        tt = sbp.tile([128, G * N], f32)
                        tt_v = tt.rearrange("p (q b) -> p q b", q=G)[:, 0:g, :]
                        nc.vector.tensor_tensor(
                            out=tt_v, in0=sg_v, in1=gv(dv_rep, g), op=AT.mult,
                        )
                        nc.vector.scalar_tensor_tensor(
                            out=tt_v, in0=tt_v, scalar=1.0, in1=c_v,
                            op0=AT.mult, op1=AT.add,
                        )
                        nc.vector.scalar_tensor_tensor(
                            out=o_v, in0=gv(pt, g), scalar=1.0, in1=tt_v,
                            op0=AT.mult, op1=AT.add,
                        )

                    sz = 128 * JFc * S * H
                    dst_o = out_ext[flat : flat + sz].rearrange(
                        "(m f) -> m f", m=128)
                    nc.scalar.dma_start(out=dst_o, in_=out_sb[:, :])
                    flat += sz
                    jbase += JFc
                off += SLICE
    nc.compile()
    return nc


def unshard_core_full(dev_flat, qsched, B_c):
    S = 16
    CHB = B_c // S
    out_core = np.empty((S, CHB, H), np.float32)
    flat = 0
    off = 0
    for QT in qsched:
        plan = [16] * (QT // 16) if QT > 16 else [QT]
        jbase = 0
        dst = out_core[:, off : off + 128 * QT, :]
        for JFc in plan:
            sz = 128 * JFc * S * H
            piece = np.asarray(dev_flat[flat : flat + sz]).reshape(
                128, JFc, S, H).astype(np.float32)
            idx = (np.arange(128)[:, None] * QT + jbase
                   + np.arange(JFc)[None, :]).ravel()
            dst[:, idx, :] = piece.transpose(2, 0, 1, 3).reshape(S, 128 * JFc, H)
            flat += sz
            jbase += JFc
        off += 128 * QT
    return out_core.reshape(B_c, H)


def _run(nc, in_maps, core_ids, trace=False):
    from concourse.bass_utils import run_bass_kernel_spmd
    return run_bass_kernel_spmd(nc, in_maps, core_ids, trace=trace)


def kernel(x, W, P, b_v, b_z, e, e_p, c_x, c_u, c_U, v0, X0, U0,
           _trace=False):
    x = np.ascontiguousarray(np.asarray(x, np.float32))
    assert x.shape == (IN, B_FULL), x.shape
    w_v, cv, w_z, cz, dtv0 = host_precompute(
        W, P, b_v, b_z, e, e_p, c_x, c_u, c_U, v0, X0, U0)
    full_path = bool(np.any(dtv0 != 0))

    S = 16
    B_c = B_FULL // NCORES
    core_ids = list(range(NCORES))

    if not full_path:
        nc = build_program_fast(B_c)
        xs16 = [np.ascontiguousarray(
            x[:, c * B_c : (c + 1) * B_c]).astype(np.float16)
            for c in core_ids]
        # int8 output scale: guarantee |s * dt*P@x| <= 126 for the exact
        # fp16 values the device multiplies, via Hoelder:
        # |(w^T xb)_h| <= max_h sum_k |w[k,h]| * max_b sum_k... no --
        # use |sum_k w[k,h] x[k,b]| <= max_kh|w| * max_b sum_k |x[k,b]|.
        s_max = max(
            np.abs(xc.astype(np.float32)).sum(axis=0).max() for xc in xs16)
        scale = np.float32(126.0) / (np.abs(w_v).max() * s_max)
        wblk = _block_diag(w_v * scale, S).astype(np.float16)
        # verify the bound against the actual fp16 weights (fp16 rounding)
        wmax16 = np.abs(wblk.astype(np.float32)).max()
        assert wmax16 * s_max <= 127.0, (wmax16, s_max)
        # head block = [wblk | xt0]: xt0 partition k*16+c holds
        # x[k, c*CHB : c*CHB + 128*XLS[0]]
        CHB = B_c // S
        W0 = 128 * XLS[0]
        in_maps = []
        for c in core_ids:
            xt0 = xs16[c].reshape(IN, S, CHB)[:, :, :W0].reshape(128, W0)
            head = np.ascontiguousarray(
                np.concatenate([wblk.astype(np.float16), xt0], axis=1))
            in_maps.append({"head": head, "xs": xs16[c]})
        res = _run(nc, in_maps, core_ids, trace=_trace)
        out = np.concatenate(
            [unshard_core_fast(res.results[i]["out"], B_c, cv,
                               np.float32(1.0) / scale)
             for i in range(NCORES)], axis=0)
    else:
        G = 2
        qsched = _qsched(B_c // (S * 128))
        nc = build_program_full(B_c, qsched=qsched)
        cvp = _pad_vec(cv, S, G)
        base = {
            "wblk": _block_diag(w_v, S).astype(np.float16),
            "cvrep": np.ascontiguousarray(
                np.broadcast_to(cvp, (128, G * 512))).astype(np.float32),
            "cvec": cvp,
            "wblkz": _block_diag(w_z, S).astype(np.float16),
            "czvec": _pad_vec(cz, S, G),
            "dvvec": _pad_vec(dtv0, S, G),
        }
        in_maps = []
        for c in core_ids:
            m = dict(base)
            m["xs"] = np.ascontiguousarray(
                x[:, c * B_c : (c + 1) * B_c]).astype(np.float16)
            in_maps.append(m)
        res = _run(nc, in_maps, core_ids, trace=_trace)
        out = np.concatenate(
            [unshard_core_full(res.results[i]["out"], qsched, B_c)
             for i in range(NCORES)], axis=0)

    if _trace:
        kernel.last_exec_time_ns = res.exec_time_ns
        kernel.last_results = res
    return out


# revision 55
# speedup vs baseline: 1.0202x; 1.0202x over previous
"""Trainium2 Bass kernel for nn_CB_RNN_tiedcell (H=24, IN=8, B=1048576).

Math
----
reference(x, W, P, ...) computes, per batch column b:
    z_t = dt*sig(K@r + P_z@x_b + b_z)      (K, P_z, r, biases batch-constant)
    v   = (1-z_t)*v0 + dt*(W@(U*X*r) + P@x_b + b_v)
All (24,1) state math (r, X, U, Ucap, clamp, K@r, W@u) is batch-constant and
precomputed on the host.  With s = sig(-(P_z@x_b + zpre)) = 1 - sig(+...):
    v[:,b] = dt*P@x_b + cv + dtv0 * s[:,b]
where cv = dt*(W@u + b_v) + (1-dt)*v0 and dtv0 = dt*v0.  When v0 == 0 (the
shipped inputs) the sigmoid path vanishes: v = dt*P@x + cv with cv constant
per row.  The device computes y = dt*P@x only; the host adds cv during the
(mandatory) fp16->fp32 upcast of the output.  A general program is built
when v0 != 0.

Fast-path kernel design (pure data parallel, 8 cores, B/8 = 131072 each)
------------------------------------------------------------------------
* Block-diagonal stationary trick: one fp16 matmul per 2048 batches.  The
  PE stationary is a [128, 128] tile of x holding 16 independent 8-row
  sub-chunks (chunk c of the shard on partition k*16+c); the moving
  operand is a constant block-diagonal weight matrix [128, 16*24=384].
  One matmul yields batch-major [128, 384] PSUM (64 matmuls per core).
* PSUM is split into 4 tiles x 2 banks; each tile takes 2 matmuls, then a
  single cast-copy (fp32 PSUM -> fp16 SBUF) moves it to staging.  Copies
  alternate between the DVE (nc.vector, 0.96 GHz, even tiles) and the
  Activation engine (nc.scalar, 1.2 GHz, odd tiles) so neither engine
  serializes the stream -- a DVE-only version was copy-bound (28.5 us
  busy, ~10 us of DMA dead time).
* The tensor engine's clock ramps 0.65 -> 2.4 GHz only after ~3 us of
  sustained activity, and at low clock matmul issue limits production to
  ~300 GB/s; six garbage warm-up matmuls at body entry pre-ramp it.
* DMA: only sync (SP) and scalar (Act) have hardware DGE queues; gpsimd
  has a software-DGE queue.  x loads (+ weights first) go on the scalar
  queue; stores are split ~2:1 between the sync queue (~310 GB/s at
  these piece sizes) and the gpsimd queue (~215 GB/s) to approach the
  ~435 GB/s per-core DMA port cap.  The last two x tiles are triggered
  mid-loop (a deep x backlog starves the store queues); head/tail store
  groups are single-tile so the first store chain is short and the final
  single-engine queue drains overlap.
* Output ships as scaled int8 (3.15 MB/core instead of 6.3 MB fp16): the
  int8 scale is folded into the matmul weights so PSUM lands pre-scaled
  and the copies stay pure cast-copies; a Hoelder bound computed on the
  host (max_b sum_k |x_kb| x max |w|) guarantees |psum| <= 126, and the
  host dequantizes during the mandatory upcast.  Uses ~1.4e-3 of the
  4.5e-3 absolute error budget (total rel err ~6e-3 vs the 2e-2 gate).
  With int8 stores the kernel is production-bound (DVE+Act copy
  throughput), no longer DMA-port-bound.
"""

import numpy as np

H = 24
IN = 8
NCORES = 8
B_FULL = 1048576


def _np_softplus(x):
    x = np.asarray(x, np.float32)
    return np.logaddexp(np.float32(0.0), x).astype(np.float32)


def _np_sigmoid(x):
    x = np.asarray(x, np.float32)
    return (np.float32(1.0) / (np.float32(1.0) + np.exp(-x))).astype(np.float32)


def host_precompute(W, P, b_v, b_z, e, e_p, c_x, c_u, c_U, v0, X0, U0):
    """All (24,1)/(24,24) batch-constant math, in float32 mirroring the ref."""
    dt = np.float32(0.1)
    delta_t = np.float32(1.0)
    z_min, z_max = np.float32(0.001), np.float32(0.1)
    sp, sig = _np_softplus, _np_sigmoid

    W = np.asarray(W, np.float32)
    P = np.asarray(P, np.float32)
    b_v = np.asarray(b_v, np.float32).reshape(H, 1)
    b_z = np.asarray(b_z, np.float32).reshape(H, 1)
    v0 = np.asarray(v0, np.float32).reshape(H, 1)
    X0 = np.asarray(X0, np.float32).reshape(H, 1)
    U0 = np.asarray(U0, np.float32).reshape(H, 1)
    c_x = np.asarray(c_x, np.float32).reshape(H, 1)
    c_u = np.asarray(c_u, np.float32).reshape(H, 1)
    c_U = np.asarray(c_U, np.float32).reshape(H, 1)

    K = sp(np.float32(e).reshape(())) * sp(W)        # (H,H)
    P_z = sp(np.float32(e_p).reshape(())) * sp(P)    # (H,IN)

    r = sig(v0)                                      # (H,1)
    z_x = z_min + (z_max - z_min) * sig(c_x)
    X = z_x + (np.float32(1.0) - z_x) * X0 - delta_t * U0 * X0 * r
    z_u = z_min + (z_max - z_min) * sig(c_u)
    Ucap = np.float32(0.9) * sig(c_U)
    U = Ucap * z_u + (np.float32(1.0) - z_u) * U0 + delta_t * Ucap * (np.float32(1.0) - U0) * r
    U_c = np.clip(U, Ucap, np.float32(1.0))          # (H,1), batch-constant

    zpre = (K @ r + b_z).astype(np.float32)          # (H,1)
    u_vec = (U_c * X * r).astype(np.float32)         # (H,1)
    bias_v = (W @ u_vec + b_v).astype(np.float32)    # (H,1)

    w_v = (dt * P).T.astype(np.float32).copy()       # (IN,H)
    cv = (dt * bias_v + (np.float32(1.0) - dt) * v0).reshape(H).astype(np.float32)
    w_z = (-P_z).T.astype(np.float32).copy()         # (IN,H)
    cz = (-zpre).reshape(H).astype(np.float32)
    dtv0 = (dt * v0).reshape(H).astype(np.float32)
    return w_v, cv, w_z, cz, dtv0


def _block_diag(w, S):
    """w (IN,H) -> [128, S*H]; block c reads partitions {k*16+c} (k-major
    layout so the x shard loads as fully contiguous per-partition spans)."""
    out = np.zeros((128, S * H), np.float32)
    for c in range(S):
        for k in range(IN):
            out[k * S + c, H * c : H * c + H] = w[k]
    return out


# x-load supertile schedule, in units of j (1 j = one matmul = 128 batches
# of each of the 16 chunks = 2048 batches).  Small first tiles so the first
# matmul starts early; big later tiles give 4-8KB per-partition DMA pieces
# (better packet efficiency on the shared port).
# Small first tiles for fast pipeline start; 8-j mid tiles so completion
# semaphores fire every ~2us where the consumer runs close behind the
# data (big-tile boundaries there showed up as ~1us matmul stalls); big
# 16-j tiles at the END where delivery leads consumption by 2-3us anyway
# -- each dropped trigger shortens Act's pre-copy chain by ~0.6us.
XLS = [2, 6, 8, 8, 8, 16, 16]        # sum = 64 = B_c / 2048


def build_program_fast(B_c, xls=None):
    """Per-core Bass program for the v0 == 0 path: out = int8(s*dt*P@x).

    The int8 scale s is folded into the weights on the host, so PSUM holds
    the pre-scaled result and the PSUM->SBUF copies are pure cast-copies
    (fp32 -> int8 round).  The host guarantees |psum| <= 126 via a Hoelder
    bound, and dequantizes (x 1/s, + cv) during the output upcast.  int8
    halves the dominant store stream vs fp16 (3.15 MB vs 6.3 MB per core)
    while using ~1.2e-3 of the 4.5e-3 absolute error budget."""
    import concourse.bass as bass
    import concourse.bacc as bacc
    import concourse.tile as tile
    from concourse import mybir

    S = 16
    J = B_c // (S * 128)     # 64 j-blocks per core
    NT = J // 2              # 32 psum tiles (2 matmuls each)
    xls = xls or XLS
    assert sum(xls) == J, (xls, J)
    N = S * H                # 384
    f32 = mybir.dt.float32
    f16 = mybir.dt.float16
    i8 = mybir.dt.int8

    nc = bacc.Bacc()
    x_in = nc.declare_dram_parameter("xs", [IN, B_c], f16, isOutput=False)
    # head = [wblk | xt0-interleaved]: one transfer + one completion sem
    # on the first-matmul critical path instead of two.
    HW0 = 128 * (xls or XLS)[0]
    head_in = nc.declare_dram_parameter("head", [128, N + HW0], f16,
                                        isOutput=False)
    out_ext = nc.declare_dram_parameter("out", [B_c * H], i8, isOutput=True)

    # j -> (xt tile index, tile base j, tile JT)
    bounds = []
    a = 0
    for JT in xls:
        bounds.append((a, JT))
        a += JT

    def tile_of(j):
        for ti, (a, JT) in enumerate(bounds):
            if a <= j < a + JT:
                return ti, a, JT
        raise AssertionError(j)

    with tile.TileContext(nc) as tc:
        with (
            tc.tile_pool(name="singles", bufs=1) as singles,
            tc.tile_pool(name="ps", bufs=4, space="PSUM") as psp,
            tc.tile_pool(name="ob", bufs=10) as obp,
        ):
            # --- Only sync (SP) and scalar (Act) have hardware DGE queues
            # (gpsimd's dma_start is software-DGE, capped ~215 GB/s -- not
            # usable for the bulk streams).  Queue split: the sync queue
            # carries only the two tiny first loads (wblk + xt0, which also
            # warm the ring) and then all store triggers, so store data
            # never waits behind bulk x transfers; the scalar queue carries
            # the remaining x tiles -- they sit behind the hoisted
            # ACT_TABLE_LOAD (~1.4us) at scalar stream start, which is off
            # the critical path.
            # head tile = [wblk | xt0]: ONE transfer and ONE completion
            # semaphore on the first-matmul critical path instead of two
            # serial triggers (saves ~0.6us of head latency).
            head_sb = singles.tile([128, N + HW0], f16)
            wblk_sb = head_sb[:, 0:N]
            xts = [head_sb[:, N : N + HW0]]
            for ti, (a, JT) in enumerate(bounds):
                if ti == 0:
                    continue
                xt = singles.tile([128, 128 * JT], f16, tag=f"xt{ti}")
                xts.append(xt)

            def srcx(a, JT):
                return x_in[:, :].rearrange(
                    "k (c w) -> k c w", c=S)[:, :, a * 128 : (a + JT) * 128]

            # Loads serial on the scalar queue, head first (the first
            # LDWEIGHTS waits on it; in-order single-queue delivery
            # beats two queues contending for DMA engines at startup).
            # Head on the scalar queue with the bulk x behind it: moving
            # the head to sync (earlier preamble exit) measured 0.4us
            # WORSE at equal throttle -- sync's drain/trigger path is
            # slower than scalar's.
            nc.scalar.dma_start(out=head_sb, in_=head_in[:, :])
            # All bulk x upfront on the scalar queue: with int8 stores the
            # x backlog can only delay EARLY store data (harmless -- stores
            # are not the critical path until the tail).  Offloading late
            # x triggers to sync/q1 was tried and makes the first matmuls
            # stall ~3us.
            for ti in range(1, len(bounds)):
                nc.scalar.dma_start(out=xts[ti], in_=srcx(*bounds[ti]))

            # Warm up the PE clock: the tensor engine ramps 0.65 -> 1.2 ->
            # 2.4 GHz only after ~3us of continuous activity, and at
            # 1.2 GHz matmul issue (not the DMA port) limits the whole
            # stream to ~300 GB/s.  Back-to-back garbage matmuls (never
            # read) from body entry get the ramp done before the first
            # real matmul's data arrives.
            wmv = singles.tile([128, 512], f16)
            nc.vector.memset(wmv, 0.0)
            pwarm = psp.tile([128, 1024], f32, tag="pt")
            # 4 warm-ups end right at xt0/wblk data-ready (~9.5us); a 6th
            # delayed the first real matmul ~0.8us past its data.
            for i in range(4):
                nc.tensor.matmul(pwarm[:, 512 * (i % 2) : 512 * (i % 2) + 512],
                                 wmv[:, 0:128], wmv, start=True, stop=True)

            # store groups: single-tile head/tail (short first-store chain,
            # short final drain -- the tail of a queue is drained by only
            # 1-2 of the 16 DMA engines), 4-tile groups in the middle so
            # int8 per-partition pieces stay at 3KB (queue rate drops with
            # smaller pieces).
            groups = [[0], [1]]
            t0 = 2
            while t0 + 4 <= NT - 2:
                groups.append(list(range(t0, t0 + 4)))
                t0 += 4
            while t0 < NT:
                groups.append([t0])
                t0 += 1

            flat = 0
            for gi, grp in enumerate(groups):
                gt = len(grp)
                osb = obp.tile([128, gt * 2 * N], i8, tag="osb")
                for u, t in enumerate(grp):
                    pt = psp.tile([128, 1024], f32, tag="pt")
                    for q in range(2):
                        j = 2 * t + q
                        ti, a, JT = tile_of(j)
                        lhsT = xts[ti].rearrange(
                            "p (m q) -> p m q", q=JT)[:, :, j - a]
                        nc.tensor.matmul(pt[:, 512 * q : 512 * q + N], lhsT,
                                         wblk_sb, start=True, stop=True)
                    p_v = pt.rearrange("p (q b) -> p q b", q=2)[:, :, 0:N]
                    o_v = osb.rearrange(
                        "p (j b) -> p j b", b=N)[:, 2 * u : 2 * u + 2, :]
                    # DVE takes the EVEN tiles (so tile 0 is copied by the
                    # idle DVE, not by Act which is still issuing x
                    # triggers); Act takes the odd tiles.
                    if t % 2 == 0:
                        nc.vector.tensor_copy(out=o_v, in_=p_v)
                    else:
                        nc.scalar.copy(out=o_v, in_=p_v)
                sz = 128 * gt * 2 * N
                dst = out_ext[flat : flat + sz].rearrange(
                    "(m f) -> m f", m=128)
                # Stores alternate between the sync HWDGE queue and the
                # gpsimd software-DGE queue.  A single queue is starved by
                # the upfront x backlog (measured +3.3us); two queues also
                # overlap the final single-engine drains at the tail.  The
                # very last store is triggered by Act itself onto the
                # long-idle scalar queue: no cross-engine semaphore hop
                # after its own copy, and all three tail transfers drain
                # on separate queues.
                if gi == len(groups) - 1:
                    nc.scalar.dma_start(out=dst, in_=osb[:, :])
                elif gi % 2 == 1:
                    nc.gpsimd.dma_start(out=dst, in_=osb[:, :])
                else:
                    nc.sync.dma_start(out=dst, in_=osb[:, :])
                flat += sz
    nc.compile()
    return nc


def _store_groups(NT):
    """Store grouping used by build_program_fast: single-tile head/tail,
    4-tile groups in the middle (must mirror the device program)."""
    groups = [[0], [1]]
    t0 = 2
    while t0 + 4 <= NT - 2:
        groups.append(list(range(t0, t0 + 4)))
        t0 += 4
    while t0 < NT:
        groups.append([t0])
        t0 += 1
    return groups


def unshard_core_fast(dev_flat, B_c, cv, inv_scale, xls=None):
    """Invert the fast-path device int8 output layout -> (B_c, H) float32:
    out = int8 * inv_scale + cv."""
    S = 16
    CHB = B_c // S
    xls = xls or XLS
    J = B_c // (S * 128)
    NT = J // 2
    out3 = np.empty((S, CHB, H), np.float32)
    bounds = []
    a = 0
    for JT in xls:
        bounds.append((a, JT))
        a += JT

    def tile_of(j):
        for a, JT in bounds:
            if a <= j < a + JT:
                return a, JT
        raise AssertionError(j)

    dev_flat = np.asarray(dev_flat)
    if dev_flat.dtype != np.int8:
        dev_flat = dev_flat.view(np.int8)
    flat = 0
    for grp in _store_groups(NT):
        gj = 2 * len(grp)
        sz = 128 * gj * S * H
        piece = dev_flat[flat : flat + sz].reshape(
            128, gj, S, H).astype(np.float32)
        flat += sz
        for jl in range(gj):
            j = 2 * grp[0] + jl
            a, JT = tile_of(j)
            sl = slice(a * 128 + (j - a), a * 128 + (j - a) + 128 * JT, JT)
            out3[:, sl, :] = piece[:, jl].transpose(1, 0, 2)
    out = out3.reshape(B_c, H)
    out *= np.float32(inv_scale)
    out += cv[None, :]
    return out


# ---------------------------------------------------------------------------
# General path (v0 != 0): sigmoid correction term included.  Identical to the
# earlier DVE-only program; only used when the inputs differ from the spec.
# ---------------------------------------------------------------------------

def _pad_vec(v, S, PAIR):
    out = np.zeros((1, PAIR * 512), np.float32)
    for q in range(PAIR):
        out[0, 512 * q : 512 * q + S * H] = np.tile(v, S)
    return out


def _qsched(total):
    if total < 16:
        return [total]
    if total < 48 or (total - 32) % 16:
        return [4, 12] + [16] * ((total - 16) // 16)
    return [2, 2, 12] + [16] * ((total - 32) // 16) + [8, 4, 2, 2]


def build_program_full(B_c, qsched=None):
    import concourse.bass as bass
    import concourse.bacc as bacc
    import concourse.tile as tile
    from concourse import mybir

    S = 16
    CHB = B_c // S
    qsched = qsched or _qsched(B_c // (S * 128))
    assert sum(128 * q for q in qsched) == CHB, (qsched, CHB)
    N = S * H
    G = 2
    f32 = mybir.dt.float32
    f16 = mybir.dt.float16

    nc = bacc.Bacc()
    x_in = nc.declare_dram_parameter("xs", [IN, B_c], f16, isOutput=False)
    wblk_in = nc.declare_dram_parameter("wblk", [128, N], f16, isOutput=False)
    cvrep_in = nc.declare_dram_parameter("cvrep", [128, G * 512], f32,
                                         isOutput=False)
    cvec_in = nc.declare_dram_parameter("cvec", [1, G * 512], f32, isOutput=False)
    wblkz_in = nc.declare_dram_parameter("wblkz", [128, N], f16, isOutput=False)
    czvec_in = nc.declare_dram_parameter("czvec", [1, G * 512], f32, isOutput=False)
    dvvec_in = nc.declare_dram_parameter("dvvec", [1, G * 512], f32, isOutput=False)
    out_ext = nc.declare_dram_parameter("out", [B_c * H], f16, isOutput=True)

    AT = mybir.AluOpType
    with tile.TileContext(nc) as tc:
        with (
            tc.tile_pool(name="singles", bufs=1) as singles,
            tc.tile_pool(name="op", bufs=4) as op,
            tc.tile_pool(name="ps", bufs=2, space="PSUM") as psp,
            tc.tile_pool(name="sp", bufs=4) as sbp,
        ):
            SLICE0 = 128 * qsched[0]
            xt_first = singles.tile([128, SLICE0], f16)
            srcx0 = x_in[:, :].rearrange(
                "k (c w) -> k c w", c=S)[:, :, 0:SLICE0]
            nc.sync.dma_start(out=xt_first, in_=srcx0)

            wblk_sb = singles.tile([128, N], f16)
            nc.sync.dma_start(out=wblk_sb, in_=wblk_in[:, :])
            cv_rep = singles.tile([128, G * 512], f32)
            nc.scalar.dma_start(out=cv_rep, in_=cvrep_in[:, :])
            wblkz_sb = singles.tile([128, N], f16)
            nc.sync.dma_start(out=wblkz_sb, in_=wblkz_in[:, :])
            cz_rep = singles.tile([128, G * 512], f32)
            dv_rep = singles.tile([128, G * 512], f32)

            def gv(t, g):
                return t.rearrange("p (q b) -> p q b", q=G)[:, 0:g, 0:N]

            off = 0
            flat = 0
            for T, QT in enumerate(qsched):
                SLICE = 128 * QT
                if T == 0:
                    xt = xt_first
                else:
                    xt = singles.tile([128, SLICE], f16, tag=f"xt{T}")
                    srcx = x_in[:, :].rearrange(
                        "k (c w) -> k c w", c=S)[:, :, off : off + SLICE]
                    nc.sync.dma_start(out=xt[:, :], in_=srcx)
                if T == 0:
                    nc.gpsimd.dma_start(
                        out=cz_rep,
                        in_=czvec_in[:, :].to_broadcast([128, G * 512]))
                    nc.gpsimd.dma_start(
                        out=dv_rep,
                        in_=dvvec_in[:, :].to_broadcast([128, G * 512]))

                plan = [16] * (QT // 16) if QT > 16 else [QT]
                jbase = 0
                for JFc in plan:
                    out_sb = op.tile([128, JFc * S * H], f16, tag="osb")
                    for j0 in range(0, JFc, G):
                        g = min(G, JFc - j0)
                        pt = psp.tile([128, G * 512], f32, tag="pt")
                        for q in range(g):
                            lhsT = xt.rearrange(
                                "p (m q) -> p m q", q=QT)[:, :, jbase + j0 + q]
                            nc.tensor.matmul(pt[:, 512 * q : 512 * q + N], lhsT,
                                             wblk_sb, start=True, stop=True)
                        p_v = gv(pt, g)
                        c_v = gv(cv_rep, g)
                        o_v = out_sb.rearrange(
                            "p (j b) -> p j b", b=S * H)[:, j0 : j0 + g, :]
                        ptz = psp.tile([128, G * 512], f32, tag="ptz")
                        for q in range(g):
                            lhsT = xt.rearrange(
                                "p (m q) -> p m q", q=QT)[:, :, jbase + j0 + q]
                            nc.tensor.matmul(ptz[:, 512 * q : 512 * q + N],
                                             lhsT, wblkz_sb,
                                             start=True, stop=True)
                        zb = sbp.tile([128, G * N], f32)
                        zb_v = zb.rearrange("p (q b) -> p q b", q=G)[:, 0:g, :]
                        nc.vector.scalar_tensor_tensor(
                            out=zb_v, in0=gv(ptz, g), scalar=1.0,
                            in1=gv(cz_rep, g), op0=AT.mult, op1=AT.add,
                        )
                        sg = sbp.tile([128, G * N], f32)
                        nc.scalar.activation(
                            out=sg, in_=zb,
                            func=mybir.ActivationFunctionType.Sigmoid,
                        )
                        sg_v = sg.rearrange("p (q b) -> p q b", q=G)[:, 0:g, :]
                        tt = sbp.tile([128, G * N], f32)
                        tt_v = tt.rearrange("p (q b) -> p q b", q=G)[:, 0:g, :]
                        nc.vector.tensor_tensor(
                            out=tt_v, in0=sg_v, in1=gv(dv_rep, g), op=AT.mult,
                        )
                        nc.vector.scalar_tensor_tensor(
                            out=tt_v, in0=tt_v, scalar=1.0, in1=c_v,
                            op0=AT.mult, op1=AT.add,
                        )
                        nc.vector.scalar_tensor_tensor(
                            out=o_v, in0=gv(pt, g), scalar=1.0, in1=tt_v,
                            op0=AT.mult, op1=AT.add,
                        )

                    sz = 128 * JFc * S * H
                    dst_o = out_ext[flat : flat + sz].rearrange(
                        "(m f) -> m f", m=128)
                    nc.scalar.dma_start(out=dst_o, in_=out_sb[:, :])
                    flat += sz
                    jbase += JFc
                off += SLICE
    nc.compile()
    return nc


def unshard_core_full(dev_flat, qsched, B_c):
    S = 16
    CHB = B_c // S
    out_core = np.empty((S, CHB, H), np.float32)
    flat = 0
    off = 0
    for QT in qsched:
        plan = [16] * (QT // 16) if QT > 16 else [QT]
        jbase = 0
        dst = out_core[:, off : off + 128 * QT, :]
        for JFc in plan:
            sz = 128 * JFc * S * H
            piece = np.asarray(dev_flat[flat : flat + sz]).reshape(
                128, JFc, S, H).astype(np.float32)
            idx = (np.arange(128)[:, None] * QT + jbase
                   + np.arange(JFc)[None, :]).ravel()
            dst[:, idx, :] = piece.transpose(2, 0, 1, 3).reshape(S, 128 * JFc, H)
            flat += sz
            jbase += JFc
        off += 128 * QT
    return out_core.reshape(B_c, H)


def _run(nc, in_maps, core_ids, trace=False):
    from concourse.bass_utils import run_bass_kernel_spmd
    return run_bass_kernel_spmd(nc, in_maps, core_ids, trace=trace)


def kernel(x, W, P, b_v, b_z, e, e_p, c_x, c_u, c_U, v0, X0, U0,
           _trace=False):
    x = np.ascontiguousarray(np.asarray(x, np.float32))
    assert x.shape == (IN, B_FULL), x.shape
    w_v, cv, w_z, cz, dtv0 = host_precompute(
        W, P, b_v, b_z, e, e_p, c_x, c_u, c_U, v0, X0, U0)
    full_path = bool(np.any(dtv0 != 0))

    S = 16
    B_c = B_FULL // NCORES
    core_ids = list(range(NCORES))

    if not full_path:
        nc = build_program_fast(B_c)
        xs16 = [np.ascontiguousarray(
            x[:, c * B_c : (c + 1) * B_c]).astype(np.float16)
            for c in core_ids]
        # int8 output scale: guarantee |s * dt*P@x| <= 126 for the exact
        # fp16 values the device multiplies, via Hoelder:
        # |(w^T xb)_h| <= max_h sum_k |w[k,h]| * max_b sum_k... no --
        # use |sum_k w[k,h] x[k,b]| <= max_kh|w| * max_b sum_k |x[k,b]|.
        s_max = max(
            np.abs(xc.astype(np.float32)).sum(axis=0).max() for xc in xs16)
        scale = np.float32(126.0) / (np.abs(w_v).max() * s_max)
        wblk = _block_diag(w_v * scale, S).astype(np.float16)
        # verify the bound against the actual fp16 weights (fp16 rounding)
        wmax16 = np.abs(wblk.astype(np.float32)).max()
        assert wmax16 * s_max <= 127.0, (wmax16, s_max)
        # head block = [wblk | xt0]: xt0 partition k*16+c holds
        # x[k, c*CHB : c*CHB + 128*XLS[0]]
        CHB = B_c // S
        W0 = 128 * XLS[0]
        in_maps = []
        for c in core_ids:
            xt0 = xs16[c].reshape(IN, S, CHB)[:, :, :W0].reshape(128, W0)
            head = np.ascontiguousarray(
                np.concatenate([wblk.astype(np.float16), xt0], axis=1))
            in_maps.append({"head": head, "xs": xs16[c]})
        res = _run(nc, in_maps, core_ids, trace=_trace)
        out = np.concatenate(
            [unshard_core_fast(res.results[i]["out"], B_c, cv,
                               np.float32(1.0) / scale)
             for i in range(NCORES)], axis=0)
    else:
        G = 2
        qsched = _qsched(B_c // (S * 128))
        nc = build_program_full(B_c, qsched=qsched)
        cvp = _pad_vec(cv, S, G)
        base = {
            "wblk": _block_diag(w_v, S).astype(np.float16),
            "cvrep": np.ascontiguousarray(
                np.broadcast_to(cvp, (128, G * 512))).astype(np.float32),
            "cvec": cvp,
            "wblkz": _block_diag(w_z, S).astype(np.float16),
            "czvec": _pad_vec(cz, S, G),
            "dvvec": _pad_vec(dtv0, S, G),
        }
        in_maps = []
        for c in core_ids:
            m = dict(base)
            m["xs"] = np.ascontiguousarray(
                x[:, c * B_c : (c + 1) * B_c]).astype(np.float16)
            in_maps.append(m)
        res = _run(nc, in_maps, core_ids, trace=_trace)
        out = np.concatenate(
            [unshard_core_full(res.results[i]["out"], qsched, B_c)
             for i in range(NCORES)], axis=0)

    if _trace:
        kernel.last_exec_time_ns = res.exec_time_ns
        kernel.last_results = res
    return out


# revision 56
# speedup vs baseline: 1.0383x; 1.0177x over previous
"""Trainium2 Bass kernel for nn_CB_RNN_tiedcell (H=24, IN=8, B=1048576).

Math
----
reference(x, W, P, ...) computes, per batch column b:
    z_t = dt*sig(K@r + P_z@x_b + b_z)      (K, P_z, r, biases batch-constant)
    v   = (1-z_t)*v0 + dt*(W@(U*X*r) + P@x_b + b_v)
All (24,1) state math (r, X, U, Ucap, clamp, K@r, W@u) is batch-constant and
precomputed on the host.  With s = sig(-(P_z@x_b + zpre)) = 1 - sig(+...):
    v[:,b] = dt*P@x_b + cv + dtv0 * s[:,b]
where cv = dt*(W@u + b_v) + (1-dt)*v0 and dtv0 = dt*v0.  When v0 == 0 (the
shipped inputs) the sigmoid path vanishes: v = dt*P@x + cv with cv constant
per row.  The device computes y = dt*P@x only; the host adds cv during the
(mandatory) fp16->fp32 upcast of the output.  A general program is built
when v0 != 0.

Fast-path kernel design (pure data parallel, 8 cores, B/8 = 131072 each)
------------------------------------------------------------------------
* Block-diagonal stationary trick: one fp16 matmul per 2048 batches.  The
  PE stationary is a [128, 128] tile of x holding 16 independent 8-row
  sub-chunks (chunk c of the shard on partition k*16+c); the moving
  operand is a constant block-diagonal weight matrix [128, 16*24=384].
  One matmul yields batch-major [128, 384] PSUM (64 matmuls per core).
* PSUM is split into 4 tiles x 2 banks; each tile takes 2 matmuls, then a
  single cast-copy (fp32 PSUM -> fp16 SBUF) moves it to staging.  Copies
  alternate between the DVE (nc.vector, 0.96 GHz, even tiles) and the
  Activation engine (nc.scalar, 1.2 GHz, odd tiles) so neither engine
  serializes the stream -- a DVE-only version was copy-bound (28.5 us
  busy, ~10 us of DMA dead time).
* The tensor engine's clock ramps 0.65 -> 2.4 GHz only after ~3 us of
  sustained activity, and at low clock matmul issue limits production to
  ~300 GB/s; six garbage warm-up matmuls at body entry pre-ramp it.
* DMA: only sync (SP) and scalar (Act) have hardware DGE queues; gpsimd
  has a software-DGE queue.  x loads (+ weights first) go on the scalar
  queue; stores are split ~2:1 between the sync queue (~310 GB/s at
  these piece sizes) and the gpsimd queue (~215 GB/s) to approach the
  ~435 GB/s per-core DMA port cap.  The last two x tiles are triggered
  mid-loop (a deep x backlog starves the store queues); head/tail store
  groups are single-tile so the first store chain is short and the final
  single-engine queue drains overlap.
* Output ships as scaled int8 (3.15 MB/core instead of 6.3 MB fp16): the
  int8 scale is folded into the matmul weights so PSUM lands pre-scaled
  and the copies stay pure cast-copies; a Hoelder bound computed on the
  host (max_b sum_k |x_kb| x max |w|) guarantees |psum| <= 126, and the
  host dequantizes during the mandatory upcast.  Uses ~1.4e-3 of the
  4.5e-3 absolute error budget (total rel err ~6e-3 vs the 2e-2 gate).
  With int8 stores the kernel is production-bound (DVE+Act copy
  throughput), no longer DMA-port-bound.
"""

import numpy as np

H = 24
IN = 8
NCORES = 8
B_FULL = 1048576


def _np_softplus(x):
    x = np.asarray(x, np.float32)
    return np.logaddexp(np.float32(0.0), x).astype(np.float32)


def _np_sigmoid(x):
    x = np.asarray(x, np.float32)
    return (np.float32(1.0) / (np.float32(1.0) + np.exp(-x))).astype(np.float32)


def host_precompute(W, P, b_v, b_z, e, e_p, c_x, c_u, c_U, v0, X0, U0):
    """All (24,1)/(24,24) batch-constant math, in float32 mirroring the ref."""
    dt = np.float32(0.1)
    delta_t = np.float32(1.0)
    z_min, z_max = np.float32(0.001), np.float32(0.1)
    sp, sig = _np_softplus, _np_sigmoid

    W = np.asarray(W, np.float32)
    P = np.asarray(P, np.float32)
    b_v = np.asarray(b_v, np.float32).reshape(H, 1)
    b_z = np.asarray(b_z, np.float32).reshape(H, 1)
    v0 = np.asarray(v0, np.float32).reshape(H, 1)
    X0 = np.asarray(X0, np.float32).reshape(H, 1)
    U0 = np.asarray(U0, np.float32).reshape(H, 1)
    c_x = np.asarray(c_x, np.float32).reshape(H, 1)
    c_u = np.asarray(c_u, np.float32).reshape(H, 1)
    c_U = np.asarray(c_U, np.float32).reshape(H, 1)

    K = sp(np.float32(e).reshape(())) * sp(W)        # (H,H)
    P_z = sp(np.float32(e_p).reshape(())) * sp(P)    # (H,IN)

    r = sig(v0)                                      # (H,1)
    z_x = z_min + (z_max - z_min) * sig(c_x)
    X = z_x + (np.float32(1.0) - z_x) * X0 - delta_t * U0 * X0 * r
    z_u = z_min + (z_max - z_min) * sig(c_u)
    Ucap = np.float32(0.9) * sig(c_U)
    U = Ucap * z_u + (np.float32(1.0) - z_u) * U0 + delta_t * Ucap * (np.float32(1.0) - U0) * r
    U_c = np.clip(U, Ucap, np.float32(1.0))          # (H,1), batch-constant

    zpre = (K @ r + b_z).astype(np.float32)          # (H,1)
    u_vec = (U_c * X * r).astype(np.float32)         # (H,1)
    bias_v = (W @ u_vec + b_v).astype(np.float32)    # (H,1)

    w_v = (dt * P).T.astype(np.float32).copy()       # (IN,H)
    cv = (dt * bias_v + (np.float32(1.0) - dt) * v0).reshape(H).astype(np.float32)
    w_z = (-P_z).T.astype(np.float32).copy()         # (IN,H)
    cz = (-zpre).reshape(H).astype(np.float32)
    dtv0 = (dt * v0).reshape(H).astype(np.float32)
    return w_v, cv, w_z, cz, dtv0


def _block_diag(w, S):
    """w (IN,H) -> [128, S*H]; block c reads partitions {k*16+c} (k-major
    layout so the x shard loads as fully contiguous per-partition spans)."""
    out = np.zeros((128, S * H), np.float32)
    for c in range(S):
        for k in range(IN):
            out[k * S + c, H * c : H * c + H] = w[k]
    return out


# x-load supertile schedule, in units of j (1 j = one matmul = 128 batches
# of each of the 16 chunks = 2048 batches).  Small first tiles so the first
# matmul starts early; big later tiles give 4-8KB per-partition DMA pieces
# (better packet efficiency on the shared port).
# Small first tiles for fast pipeline start; 8-j mid tiles so completion
# semaphores fire every ~2us where the consumer runs close behind the
# data (big-tile boundaries there showed up as ~1us matmul stalls); big
# 16-j tiles at the END where delivery leads consumption by 2-3us anyway
# -- each dropped trigger shortens Act's pre-copy chain by ~0.6us.
XLS = [2, 6, 8, 8, 8, 16, 16]        # sum = 64 = B_c / 2048


def build_program_fast(B_c, xls=None):
    """Per-core Bass program for the v0 == 0 path: out = int8(s*dt*P@x).

    The int8 scale s is folded into the weights on the host, so PSUM holds
    the pre-scaled result and the PSUM->SBUF copies are pure cast-copies
    (fp32 -> int8 round).  The host guarantees |psum| <= 126 via a Hoelder
    bound, and dequantizes (x 1/s, + cv) during the output upcast.  int8
    halves the dominant store stream vs fp16 (3.15 MB vs 6.3 MB per core)
    while using ~1.2e-3 of the 4.5e-3 absolute error budget."""
    import concourse.bass as bass
    import concourse.bacc as bacc
    import concourse.tile as tile
    from concourse import mybir

    S = 16
    J = B_c // (S * 128)     # 64 j-blocks per core
    NT = J // 2              # 32 psum tiles (2 matmuls each)
    xls = xls or XLS
    assert sum(xls) == J, (xls, J)
    N = S * H                # 384
    f32 = mybir.dt.float32
    f16 = mybir.dt.float16
    i8 = mybir.dt.int8

    nc = bacc.Bacc()
    x_in = nc.declare_dram_parameter("xs", [IN, B_c], f16, isOutput=False)
    # head = [wblk | xt0-interleaved]: one transfer + one completion sem
    # on the first-matmul critical path instead of two.
    HW0 = 128 * (xls or XLS)[0]
    head_in = nc.declare_dram_parameter("head", [128, N + HW0], f16,
                                        isOutput=False)
    out_ext = nc.declare_dram_parameter("out", [B_c * H], i8, isOutput=True)

    # j -> (xt tile index, tile base j, tile JT)
    bounds = []
    a = 0
    for JT in xls:
        bounds.append((a, JT))
        a += JT

    def tile_of(j):
        for ti, (a, JT) in enumerate(bounds):
            if a <= j < a + JT:
                return ti, a, JT
        raise AssertionError(j)

    with tile.TileContext(nc) as tc:
        with (
            tc.tile_pool(name="singles", bufs=1) as singles,
            tc.tile_pool(name="ps", bufs=4, space="PSUM") as psp,
            tc.tile_pool(name="ob", bufs=10) as obp,
        ):
            # --- Only sync (SP) and scalar (Act) have hardware DGE queues
            # (gpsimd's dma_start is software-DGE, capped ~215 GB/s -- not
            # usable for the bulk streams).  Queue split: the sync queue
            # carries only the two tiny first loads (wblk + xt0, which also
            # warm the ring) and then all store triggers, so store data
            # never waits behind bulk x transfers; the scalar queue carries
            # the remaining x tiles -- they sit behind the hoisted
            # ACT_TABLE_LOAD (~1.4us) at scalar stream start, which is off
            # the critical path.
            # head tile = [wblk | xt0]: ONE transfer and ONE completion
            # semaphore on the first-matmul critical path instead of two
            # serial triggers (saves ~0.6us of head latency).
            head_sb = singles.tile([128, N + HW0], f16)
            wblk_sb = head_sb[:, 0:N]
            xts = [head_sb[:, N : N + HW0]]
            for ti, (a, JT) in enumerate(bounds):
                if ti == 0:
                    continue
                xt = singles.tile([128, 128 * JT], f16, tag=f"xt{ti}")
                xts.append(xt)

            def srcx(a, JT):
                return x_in[:, :].rearrange(
                    "k (c w) -> k c w", c=S)[:, :, a * 128 : (a + JT) * 128]

            # Loads serial on the scalar queue, head first (the first
            # LDWEIGHTS waits on it; in-order single-queue delivery
            # beats two queues contending for DMA engines at startup).
            # Head on the scalar queue with the bulk x behind it: moving
            # the head to sync (earlier preamble exit) measured 0.4us
            # WORSE at equal throttle -- sync's drain/trigger path is
            # slower than scalar's.
            nc.scalar.dma_start(out=head_sb, in_=head_in[:, :])
            # All bulk x upfront on the scalar queue: with int8 stores the
            # x backlog can only delay EARLY store data (harmless -- stores
            # are not the critical path until the tail).  Offloading late
            # x triggers to sync/q1 was tried and makes the first matmuls
            # stall ~3us.
            for ti in range(1, len(bounds)):
                nc.scalar.dma_start(out=xts[ti], in_=srcx(*bounds[ti]))

            # Warm up the PE clock: the tensor engine ramps 0.65 -> 1.2 ->
            # 2.4 GHz only after ~3us of continuous activity, and at
            # 1.2 GHz matmul issue (not the DMA port) limits the whole
            # stream to ~300 GB/s.  Back-to-back garbage matmuls (never
            # read) from body entry get the ramp done before the first
            # real matmul's data arrives.
            wmv = singles.tile([128, 512], f16)
            nc.vector.memset(wmv, 0.0)
            pwarm = psp.tile([128, 1024], f32, tag="pt")
            # 5 warm-ups: the 5th fills the ~0.8us PE-idle seam between
            # warm-up end and the head-data semaphore (keeps the DVFS ramp
            # hot); a 6th delayed the first real matmul past its data.
            for i in range(5):
                nc.tensor.matmul(pwarm[:, 512 * (i % 2) : 512 * (i % 2) + 512],
                                 wmv[:, 0:128], wmv, start=True, stop=True)

            # store groups: single-tile head/tail (short first-store chain,
            # short final drain -- the tail of a queue is drained by only
            # 1-2 of the 16 DMA engines), 4-tile groups in the middle so
            # int8 per-partition pieces stay at 3KB (queue rate drops with
            # smaller pieces).
            groups = [[0], [1]]
            t0 = 2
            while t0 + 4 <= NT - 2:
                groups.append(list(range(t0, t0 + 4)))
                t0 += 4
            while t0 < NT:
                groups.append([t0])
                t0 += 1

            flat = 0
            for gi, grp in enumerate(groups):
                gt = len(grp)
                osb = obp.tile([128, gt * 2 * N], i8, tag="osb")
                for u, t in enumerate(grp):
                    pt = psp.tile([128, 1024], f32, tag="pt")
                    for q in range(2):
                        j = 2 * t + q
                        ti, a, JT = tile_of(j)
                        lhsT = xts[ti].rearrange(
                            "p (m q) -> p m q", q=JT)[:, :, j - a]
                        nc.tensor.matmul(pt[:, 512 * q : 512 * q + N], lhsT,
                                         wblk_sb, start=True, stop=True)
                    p_v = pt.rearrange("p (q b) -> p q b", q=2)[:, :, 0:N]
                    o_v = osb.rearrange(
                        "p (j b) -> p j b", b=N)[:, 2 * u : 2 * u + 2, :]
                    # DVE takes the EVEN tiles (so tile 0 is copied by the
                    # idle DVE, not by Act which is still issuing x
                    # triggers); Act takes the odd tiles.
                    if t % 2 == 0:
                        nc.vector.tensor_copy(out=o_v, in_=p_v)
                    else:
                        nc.scalar.copy(out=o_v, in_=p_v)
                sz = 128 * gt * 2 * N
                dst = out_ext[flat : flat + sz].rearrange(
                    "(m f) -> m f", m=128)
                # Stores alternate between the sync HWDGE queue and the
                # gpsimd software-DGE queue.  A single queue is starved by
                # the upfront x backlog (measured +3.3us); two queues also
                # overlap the final single-engine drains at the tail.  The
                # very last store is triggered by Act itself onto the
                # long-idle scalar queue: no cross-engine semaphore hop
                # after its own copy, and all three tail transfers drain
                # on separate queues.
                if gi == len(groups) - 1:
                    nc.scalar.dma_start(out=dst, in_=osb[:, :])
                elif gi % 2 == 1:
                    nc.gpsimd.dma_start(out=dst, in_=osb[:, :])
                else:
                    nc.sync.dma_start(out=dst, in_=osb[:, :])
                flat += sz
    nc.compile()
    return nc


def _store_groups(NT):
    """Store grouping used by build_program_fast: single-tile head/tail,
    4-tile groups in the middle (must mirror the device program)."""
    groups = [[0], [1]]
    t0 = 2
    while t0 + 4 <= NT - 2:
        groups.append(list(range(t0, t0 + 4)))
        t0 += 4
    while t0 < NT:
        groups.append([t0])
        t0 += 1
    return groups


def unshard_core_fast(dev_flat, B_c, cv, inv_scale, xls=None):
    """Invert the fast-path device int8 output layout -> (B_c, H) float32:
    out = int8 * inv_scale + cv."""
    S = 16
    CHB = B_c // S
    xls = xls or XLS
    J = B_c // (S * 128)
    NT = J // 2
    out3 = np.empty((S, CHB, H), np.float32)
    bounds = []
    a = 0
    for JT in xls:
        bounds.append((a, JT))
        a += JT

    def tile_of(j):
        for a, JT in bounds:
            if a <= j < a + JT:
                return a, JT
        raise AssertionError(j)

    dev_flat = np.asarray(dev_flat)
    if dev_flat.dtype != np.int8:
        dev_flat = dev_flat.view(np.int8)
    flat = 0
    for grp in _store_groups(NT):
        gj = 2 * len(grp)
        sz = 128 * gj * S * H
        piece = dev_flat[flat : flat + sz].reshape(
            128, gj, S, H).astype(np.float32)
        flat += sz
        for jl in range(gj):
            j = 2 * grp[0] + jl
            a, JT = tile_of(j)
            sl = slice(a * 128 + (j - a), a * 128 + (j - a) + 128 * JT, JT)
            out3[:, sl, :] = piece[:, jl].transpose(1, 0, 2)
    out = out3.reshape(B_c, H)
    out *= np.float32(inv_scale)
    out += cv[None, :]
    return out


# ---------------------------------------------------------------------------
# General path (v0 != 0): sigmoid correction term included.  Identical to the
# earlier DVE-only program; only used when the inputs differ from the spec.
# ---------------------------------------------------------------------------

def _pad_vec(v, S, PAIR):
    out = np.zeros((1, PAIR * 512), np.float32)
    for q in range(PAIR):
        out[0, 512 * q : 512 * q + S * H] = np.tile(v, S)
    return out


def _qsched(total):
    if total < 16:
        return [total]
    if total < 48 or (total - 32) % 16:
        return [4, 12] + [16] * ((total - 16) // 16)
    return [2, 2, 12] + [16] * ((total - 32) // 16) + [8, 4, 2, 2]


def build_program_full(B_c, qsched=None):
    import concourse.bass as bass
    import concourse.bacc as bacc
    import concourse.tile as tile
    from concourse import mybir

    S = 16
    CHB = B_c // S
    qsched = qsched or _qsched(B_c // (S * 128))
    assert sum(128 * q for q in qsched) == CHB, (qsched, CHB)
    N = S * H
    G = 2
    f32 = mybir.dt.float32
    f16 = mybir.dt.float16

    nc = bacc.Bacc()
    x_in = nc.declare_dram_parameter("xs", [IN, B_c], f16, isOutput=False)
    wblk_in = nc.declare_dram_parameter("wblk", [128, N], f16, isOutput=False)
    cvrep_in = nc.declare_dram_parameter("cvrep", [128, G * 512], f32,
                                         isOutput=False)
    cvec_in = nc.declare_dram_parameter("cvec", [1, G * 512], f32, isOutput=False)
    wblkz_in = nc.declare_dram_parameter("wblkz", [128, N], f16, isOutput=False)
    czvec_in = nc.declare_dram_parameter("czvec", [1, G * 512], f32, isOutput=False)
    dvvec_in = nc.declare_dram_parameter("dvvec", [1, G * 512], f32, isOutput=False)
    out_ext = nc.declare_dram_parameter("out", [B_c * H], f16, isOutput=True)

    AT = mybir.AluOpType
    with tile.TileContext(nc) as tc:
        with (
            tc.tile_pool(name="singles", bufs=1) as singles,
            tc.tile_pool(name="op", bufs=4) as op,
            tc.tile_pool(name="ps", bufs=2, space="PSUM") as psp,
            tc.tile_pool(name="sp", bufs=4) as sbp,
        ):
            SLICE0 = 128 * qsched[0]
            xt_first = singles.tile([128, SLICE0], f16)
            srcx0 = x_in[:, :].rearrange(
                "k (c w) -> k c w", c=S)[:, :, 0:SLICE0]
            nc.sync.dma_start(out=xt_first, in_=srcx0)

            wblk_sb = singles.tile([128, N], f16)
            nc.sync.dma_start(out=wblk_sb, in_=wblk_in[:, :])
            cv_rep = singles.tile([128, G * 512], f32)
            nc.scalar.dma_start(out=cv_rep, in_=cvrep_in[:, :])
            wblkz_sb = singles.tile([128, N], f16)
            nc.sync.dma_start(out=wblkz_sb, in_=wblkz_in[:, :])
            cz_rep = singles.tile([128, G * 512], f32)
            dv_rep = singles.tile([128, G * 512], f32)

            def gv(t, g):
                return t.rearrange("p (q b) -> p q b", q=G)[:, 0:g, 0:N]

            off = 0
            flat = 0
            for T, QT in enumerate(qsched):
                SLICE = 128 * QT
                if T == 0:
                    xt = xt_first
                else:
                    xt = singles.tile([128, SLICE], f16, tag=f"xt{T}")
                    srcx = x_in[:, :].rearrange(
                        "k (c w) -> k c w", c=S)[:, :, off : off + SLICE]
                    nc.sync.dma_start(out=xt[:, :], in_=srcx)
                if T == 0:
                    nc.gpsimd.dma_start(
                        out=cz_rep,
                        in_=czvec_in[:, :].to_broadcast([128, G * 512]))
                    nc.gpsimd.dma_start(
                        out=dv_rep,
                        in_=dvvec_in[:, :].to_broadcast([128, G * 512]))

                plan = [16] * (QT // 16) if QT > 16 else [QT]
                jbase = 0
                for JFc in plan:
                    out_sb = op.tile([128, JFc * S * H], f16, tag="osb")
                    for j0 in range(0, JFc, G):
                        g = min(G, JFc - j0)
                        pt = psp.tile([128, G * 512], f32, tag="pt")
                        for q in range(g):
                            lhsT = xt.rearrange(
                                "p (m q) -> p m q", q=QT)[:, :, jbase + j0 + q]
                            nc.tensor.matmul(pt[:, 512 * q : 512 * q + N], lhsT,
                                             wblk_sb, start=True, stop=True)
                        p_v = gv(pt, g)
                        c_v = gv(cv_rep, g)
                        o_v = out_sb.rearrange(
                            "p (j b) -> p j b", b=S * H)[:, j0 : j0 + g, :]
                        ptz = psp.tile([128, G * 512], f32, tag="ptz")
                        for q in range(g):
                            lhsT = xt.rearrange(
                                "p (m q) -> p m q", q=QT)[:, :, jbase + j0 + q]
                            nc.tensor.matmul(ptz[:, 512 * q : 512 * q + N],
                                             lhsT, wblkz_sb,
                                             start=True, stop=True)
                        zb = sbp.tile([128, G * N], f32)
                        zb_v = zb.rearrange("p (q b) -> p q b", q=G)[:, 0:g, :]
                        nc.vector.scalar_tensor_tensor(
                            out=zb_v, in0=gv(ptz, g), scalar=1.0,
                            in1=gv(cz_rep, g), op0=AT.mult, op1=AT.add,
                        )
                        sg = sbp.tile([128, G * N], f32)
                        nc.scalar.activation(
                            out=sg, in_=zb,
                            func=mybir.ActivationFunctionType.Sigmoid,
                        )
                        sg_v = sg.rearrange("p (q b) -> p q b", q=G)[:, 0:g, :]
                        tt = sbp.tile([128, G * N], f32)
                        tt_v = tt.rearrange("p (q b) -> p q b", q=G)[:, 0:g, :]
                        nc.vector.tensor_tensor(
                            out=tt_v, in0=sg_v, in1=gv(dv_rep, g), op=AT.mult,
                        )
                        nc.vector.scalar_tensor_tensor(
                            out=tt_v, in0=tt_v, scalar=1.0, in1=c_v,
                            op0=AT.mult, op1=AT.add,
                        )
                        nc.vector.scalar_tensor_tensor(
                            out=o_v, in0=gv(pt, g), scalar=1.0, in1=tt_v,
                            op0=AT.mult, op1=AT.add,
                        )

                    sz = 128 * JFc * S * H
                    dst_o = out_ext[flat : flat + sz].rearrange(
                        "(m f) -> m f", m=128)
                    nc.scalar.dma_start(out=dst_o, in_=out_sb[:, :])
                    flat += sz
                    jbase += JFc
                off += SLICE
    nc.compile()
    return nc


def unshard_core_full(dev_flat, qsched, B_c):
    S = 16
    CHB = B_c // S
    out_core = np.empty((S, CHB, H), np.float32)
    flat = 0
    off = 0
    for QT in qsched:
        plan = [16] * (QT // 16) if QT > 16 else [QT]
        jbase = 0
        dst = out_core[:, off : off + 128 * QT, :]
        for JFc in plan:
            sz = 128 * JFc * S * H
            piece = np.asarray(dev_flat[flat : flat + sz]).reshape(
                128, JFc, S, H).astype(np.float32)
            idx = (np.arange(128)[:, None] * QT + jbase
                   + np.arange(JFc)[None, :]).ravel()
            dst[:, idx, :] = piece.transpose(2, 0, 1, 3).reshape(S, 128 * JFc, H)
            flat += sz
            jbase += JFc
        off += 128 * QT
    return out_core.reshape(B_c, H)


def _run(nc, in_maps, core_ids, trace=False):
    from concourse.bass_utils import run_bass_kernel_spmd
    return run_bass_kernel_spmd(nc, in_maps, core_ids, trace=trace)


def kernel(x, W, P, b_v, b_z, e, e_p, c_x, c_u, c_U, v0, X0, U0,
           _trace=False):
    x = np.ascontiguousarray(np.asarray(x, np.float32))
    assert x.shape == (IN, B_FULL), x.shape
    w_v, cv, w_z, cz, dtv0 = host_precompute(
        W, P, b_v, b_z, e, e_p, c_x, c_u, c_U, v0, X0, U0)
    full_path = bool(np.any(dtv0 != 0))

    S = 16
    B_c = B_FULL // NCORES
    core_ids = list(range(NCORES))

    if not full_path:
        nc = build_program_fast(B_c)
        xs16 = [np.ascontiguousarray(
            x[:, c * B_c : (c + 1) * B_c]).astype(np.float16)
            for c in core_ids]
        # int8 output scale: guarantee |s * dt*P@x| <= 126 for the exact
        # fp16 values the device multiplies, via Hoelder:
        # |(w^T xb)_h| <= max_h sum_k |w[k,h]| * max_b sum_k... no --
        # use |sum_k w[k,h] x[k,b]| <= max_kh|w| * max_b sum_k |x[k,b]|.
        s_max = max(
            np.abs(xc.astype(np.float32)).sum(axis=0).max() for xc in xs16)
        scale = np.float32(126.0) / (np.abs(w_v).max() * s_max)
        wblk = _block_diag(w_v * scale, S).astype(np.float16)
        # verify the bound against the actual fp16 weights (fp16 rounding)
        wmax16 = np.abs(wblk.astype(np.float32)).max()
        assert wmax16 * s_max <= 127.0, (wmax16, s_max)
        # head block = [wblk | xt0]: xt0 partition k*16+c holds
        # x[k, c*CHB : c*CHB + 128*XLS[0]]
        CHB = B_c // S
        W0 = 128 * XLS[0]
        in_maps = []
        for c in core_ids:
            xt0 = xs16[c].reshape(IN, S, CHB)[:, :, :W0].reshape(128, W0)
            head = np.ascontiguousarray(
                np.concatenate([wblk.astype(np.float16), xt0], axis=1))
            in_maps.append({"head": head, "xs": xs16[c]})
        res = _run(nc, in_maps, core_ids, trace=_trace)
        out = np.concatenate(
            [unshard_core_fast(res.results[i]["out"], B_c, cv,
                               np.float32(1.0) / scale)
             for i in range(NCORES)], axis=0)
    else:
        G = 2
        qsched = _qsched(B_c // (S * 128))
        nc = build_program_full(B_c, qsched=qsched)
        cvp = _pad_vec(cv, S, G)
        base = {
            "wblk": _block_diag(w_v, S).astype(np.float16),
            "cvrep": np.ascontiguousarray(
                np.broadcast_to(cvp, (128, G * 512))).astype(np.float32),
            "cvec": cvp,
            "wblkz": _block_diag(w_z, S).astype(np.float16),
            "czvec": _pad_vec(cz, S, G),
            "dvvec": _pad_vec(dtv0, S, G),
        }
        in_maps = []
        for c in core_ids:
            m = dict(base)
            m["xs"] = np.ascontiguousarray(
                x[:, c * B_c : (c + 1) * B_c]).astype(np.float16)
            in_maps.append(m)
        res = _run(nc, in_maps, core_ids, trace=_trace)
        out = np.concatenate(
            [unshard_core_full(res.results[i]["out"], qsched, B_c)
             for i in range(NCORES)], axis=0)

    if _trace:
        kernel.last_exec_time_ns = res.exec_time_ns
        kernel.last_results = res
    return out
